# revision 18
# baseline (speedup 1.0000x reference)
"""Causal multi-head attention (B=2, S=2048, D=1024, 16 heads x 64) on 8
Trainium2 NeuronCores.

Sharding: batch x head-group — each core owns one batch (b = core//4) and 4
heads (4*(core%4)..+3), handled as two head-pairs hp in {0,1}. Each core gets
its batch's activations (pre-transposed, bf16) and its 4 heads' weights,
computes q/k/v, causal flash-style attention, and a partial output
projection; the host sums 4 partials per batch and adds b_O.

Device algorithm per core (all matmuls bf16 with fp32 PSUM accumulate):
  - QKV per (pp, hp) block: {q,k,v}T[128 = 2*64 headdims, tok] = W.T @ xT over
    8 contraction chunks. q/k bias+cast on DVE (tensor_scalar_add), v on ACT.
  - V re-laid-out to [pos, headdim] tiles via DMA xbar transposes (frees DVE),
    with a ones column appended so the attention-value matmul also produces
    the softmax denominator for free.
  - scores transposed (key pos on partitions); the two heads' K=64 matmuls
    row-tile into opposite halves of the PE array and run concurrently.
  - causal diagonal restriction: for diagonal key-tiles only the valid query
    sub-range is computed/exp'd/accumulated, and masking reduces to one
    [128,128] triangle multiply per head.
  - exp on ACT straight out of PSUM (scores are O(1); no max subtraction).
  - 1/sum via exp(-ln(sum)) on ACT, broadcast across partitions with a K=2
    matmul, applied while evacuating z.
  - out projection contracts both head-pairs (2x128 dims) into one PSUM tile.
  - QKV blocks and attention units are emission-interleaved so the PE-bound
    QKV stream and the ACT-bound softmax stream overlap.
"""

import functools
from collections import deque

import numpy as np
import ml_dtypes

import concourse.bass as bass
import concourse.tile as tile
import concourse.mybir as mybir
from concourse.bass_utils import run_bass_kernel_spmd

# ---------------------------------------------------------------- wait fix
# This container's walrus accepts at most ONE sync-wait per instruction
# (two for EventSemaphore); Tile emits several. Hoist the excess onto NoOps
# inserted just before the over-subscribed instruction on the same engine.
import json as _json

_WAIT_CAP = {"EventSemaphore": 2}


def _split_waits(doc):
    n = [0]

    def fix_block(block):
        insts = block.get("instructions")
        if not isinstance(insts, list):
            return
        out = []
        for inst in insts:
            si = inst.get("sync_info")
            waits = si.get("on_wait") if si else None
            cap = _WAIT_CAP.get(inst.get("opcode"), 1)
            if waits and len(waits) > cap:
                for w in waits[cap:]:
                    n[0] += 1
                    out.append(
                        {
                            "name": f"WSPL-{n[0]}",
                            "opcode": "NoOp",
                            "engine": inst["engine"],
                            "ins": [],
                            "outs": [],
                            "sync_info": {"on_wait": [w], "on_update": []},
                        }
                    )
                si["on_wait"] = waits[:cap]
            out.append(inst)
        block["instructions"] = out

    def walk(o):
        if isinstance(o, dict):
            if "instructions" in o:
                fix_block(o)
            for v in o.values():
                walk(v)
        elif isinstance(o, list):
            for v in o:
                walk(v)

    walk(doc)
    return doc


_waitfix_done = False


def _install_waitfix():
    global _waitfix_done
    if _waitfix_done:
        return
    _waitfix_done = True
    orig = bass.Bass.to_json_bytes

    def to_json_bytes(self, *a, **kw):
        doc = _json.loads(orig(self, *a, **kw))
        return _json.dumps(_split_waits(doc)).encode()

    bass.Bass.to_json_bytes = to_json_bytes


# ---------------------------------------------------------------- constants
B, S, D = 2, 2048, 1024
NHEAD, HDIM = 16, 64
NCORES = 8
SC = S  # tokens per core (one batch)
NHP = 2  # head-pairs per core (4 heads)
SCALE = 1.0 / 8.0  # 1/sqrt(HDIM)

bf16 = mybir.dt.bfloat16
f32 = mybir.dt.float32
AF = mybir.ActivationFunctionType

NDC = D // 128  # 8 contraction chunks
NKT = SC // 128  # 16 key tiles
NQB = SC // 512  # 4 query blocks
VT = NHP * SC  # 4096 virtual token-cols (hp-major)


def _build_nc():
    nc = bass.Bass()
    xT = nc.dram_tensor("xT", [D, SC], bf16, kind="ExternalInput")
    wqkv = nc.dram_tensor("wqkv", [128, NHP * NDC * 384], bf16, kind="ExternalInput")
    bqkv = nc.dram_tensor("bqkv", [128, NHP * 3], f32, kind="ExternalInput")
    wo = nc.dram_tensor("wo", [128, NHP * D], bf16, kind="ExternalInput")
    tri = nc.dram_tensor("tri", [128, 128], bf16, kind="ExternalInput")
    ones1 = nc.dram_tensor("ones1", [2, 128], bf16, kind="ExternalInput")
    outp = nc.dram_tensor("outp", [SC, D], bf16, kind="ExternalOutput")

    with tile.TileContext(nc) as tc:
        with (
            tc.tile_pool(name="const", bufs=1) as const,
            tc.tile_pool(name="attn", bufs=8) as attnp,
            tc.tile_pool(name="obuf", bufs=4) as obufp,
            tc.tile_pool(name="small", bufs=4) as small,
            tc.tile_pool(name="psum", bufs=2, space="PSUM") as psum,
        ):
            # ---- constant tiles
            w_sb = const.tile([128, NHP, 3, NDC, 128], bf16)
            xt_sb = const.tile([128, NDC, SC], bf16)
            bias_sb = const.tile([128, NHP * 3], f32)
            wo_sb = const.tile([128, NHP, D], bf16)
            tri_sb = const.tile([128, 128], bf16)
            ee_sb = const.tile([2, 128], bf16)

            qT = const.tile([128, VT], bf16)
            kT = const.tile([128, VT], bf16)
            vT = const.tile([128, VT], bf16)
            zT = const.tile([128, VT], bf16)

            # v in [pos, headdim] layout, one per head; col 64 = ones so the
            # AV matmul produces softmax denominators; padded to 80 cols for
            # the 32B-aligned DMA-transpose destination stride.
            v_sb = []
            for i in range(4):
                v = const.tile([128, NKT, 80], bf16, name=f"v_sb{i}")
                nc.gpsimd.memset(v[:, :, 64], 1.0)
                v_sb.append(v)

            wq_r = wqkv[:].rearrange("p (h g a c) -> p h g a c", h=NHP, g=3, a=NDC)
            xTr = xT[:].rearrange("(a p) m -> p a m", p=128)

            # ---- DMA order = first-use order (hp0's weights contiguous,
            # then the x chunks the first QKV block reads, etc.)
            nc.sync.dma_start(w_sb[:, 0, 2], wq_r[:, 0, 2])
            nc.sync.dma_start(xt_sb[:, 0:2, 0:512], xTr[:, 0:2, 0:512])
            nc.sync.dma_start(xt_sb[:, 2:4, 0:512], xTr[:, 2:4, 0:512])
            nc.sync.dma_start(xt_sb[:, 4:6, 0:512], xTr[:, 4:6, 0:512])
            nc.sync.dma_start(xt_sb[:, 6:8, 0:512], xTr[:, 6:8, 0:512])
            nc.sync.dma_start(xt_sb[:, :, 512:1024], xTr[:, :, 512:1024])
            nc.sync.dma_start(w_sb[:, 0, 0:2], wq_r[:, 0, 0:2])
            nc.sync.dma_start(bias_sb[:], bqkv[:])
            nc.sync.dma_start(w_sb[:, 1], wq_r[:, 1])
            nc.sync.dma_start(ee_sb[:], ones1[:])
            nc.sync.dma_start(tri_sb[:], tri[:])
            nc.sync.dma_start(xt_sb[:, :, 1024:1536], xTr[:, :, 1024:1536])
            nc.sync.dma_start(xt_sb[:, :, 1536:2048], xTr[:, :, 1536:2048])
            wo_r = wo[:].rearrange("p (h d) -> p h d", h=NHP)
            nc.sync.dma_start(wo_sb[:], wo_r)

            # ---- deferred-work queues: each unit's norm stages go on a
            # high-priority queue pumped early in the next emission block
            # (stage b late enough that its PE matmul never stalls on the
            # ln/exp chain); output projections pump from a background queue
            # gated on their query block's norms having been emitted.
            normq = deque()
            opq = deque()
            normb_done = set()

            def pump_norm():
                if normq:
                    normq.popleft()()

            def pump_op():
                if opq and opq[0][0] in normb_done:
                    opq.popleft()[1]()

            # ---- QKV projection for one (pp, hp): 1024 tokens, one
            # head-pair; V group first so its re-layout transposes start
            # early.
            def emit_qkv(pp, hp, half_major=False):
                gs = (2, 0, 1)  # V, Q, K
                pst = {
                    g: psum.tile(
                        [128, 1024], f32, tag="sc", bufs=3, name=f"qkv_ps{g}"
                    )
                    for g in gs
                }

                def mms(g, half):
                    pt = 2 * pp + half
                    dst = pst[g][:, 512 * half : 512 * half + 512]
                    for di in range(NDC):
                        nc.tensor.matmul(
                            dst,
                            w_sb[:, hp, g, di, :],
                            xt_sb[:, di, 512 * pt : 512 * pt + 512],
                            start=(di == 0),
                            stop=(di == NDC - 1),
                        )

                if half_major:
                    # first block: run all groups on the first 512-token slab
                    # so compute starts before the second slab's DMA lands
                    for g in gs:
                        mms(g, 0)
                for g in gs:
                    if half_major:
                        mms(g, 1)
                    else:
                        mms(g, 0)
                        mms(g, 1)
                    ps = pst[g]
                    dc = SC * hp + 1024 * pp
                    bcol = 3 * hp + g
                    if g == 2:
                        # v: bias+cast on ACT, then xbar-transpose each
                        # 128-token tile into [pos, headdim] layout
                        nc.scalar.activation(
                            vT[:, dc : dc + 1024],
                            ps[:],
                            AF.Identity,
                            bias=bias_sb[:, bcol : bcol + 1],
                            scale=1.0,
                        )
                        # one batched xbar transpose per head: [64, 1024] ->
                        # [128 pos, 8 tiles, 64] (row r of the transpose maps
                        # to (tile r//128, partition r%128))
                        for h in range(2):
                            nc.sync.dma_start_transpose(
                                v_sb[2 * hp + h][:, 8 * pp : 8 * pp + 8, 0:64],
                                vT[64 * h : 64 * h + 64, dc : dc + 1024],
                            )
                    else:
                        tgt = qT if g == 0 else kT
                        nc.vector.tensor_scalar_add(
                            tgt[:, dc : dc + 1024],
                            ps[:],
                            bias_sb[:, bcol : bcol + 1],
                        )
                    if g == 0:
                        pump_op()
                    else:
                        pump_norm()

            op_stage = {}

            def emit_outproj_h0(qtg):
                # hp0 half of the last query block's output projection,
                # staged to SBUF fp32 so only hp1's half remains at the tail
                op = psum.tile([128, 1024], f32, tag="sc", bufs=3, name="oph0")
                for dh in range(2):
                    nc.tensor.matmul(
                        op[:, 512 * dh : 512 * dh + 512],
                        zT[:, 128 * qtg : 128 * qtg + 128],
                        wo_sb[:, 0, 512 * dh : 512 * dh + 512],
                        start=True,
                        stop=True,
                    )
                stg = obufp.tile([128, 1024], f32, tag="stg", bufs=4, name="stg")
                nc.vector.tensor_copy(stg[:], op[:])
                op_stage[qtg] = stg

            def emit_outproj_h1_tail(qtg, tail_idx):
                op = psum.tile([128, 1024], f32, tag="sc", bufs=3, name="oph1")
                for dh in range(2):
                    nc.tensor.matmul(
                        op[:, 512 * dh : 512 * dh + 512],
                        zT[:, SC + 128 * qtg : SC + 128 * qtg + 128],
                        wo_sb[:, 1, 512 * dh : 512 * dh + 512],
                        start=True,
                        stop=True,
                    )
                ob = obufp.tile([128, 1024], bf16, name="ob")
                nc.vector.tensor_add(ob[:], op[:], op_stage[qtg][:])
                nc.sync.dma_start(outp[128 * qtg : 128 * qtg + 128, :], ob[:])

            # ---- output projection for one 128-token tile; contracts both
            # head-pairs' z into one PSUM accumulation.
            def emit_outproj_qt(qtg, tail=False):
                op = psum.tile([128, 1024], f32, tag="sc", bufs=3, name="op")
                for dh in range(2):
                    for hp in range(NHP):
                        nc.tensor.matmul(
                            op[:, 512 * dh : 512 * dh + 512],
                            zT[:, SC * hp + 128 * qtg : SC * hp + 128 * qtg + 128],
                            wo_sb[:, hp, 512 * dh : 512 * dh + 512],
                            start=(hp == 0),
                            stop=(hp == NHP - 1),
                        )
                ob = obufp.tile([128, 1024], bf16, name="ob")
                if tail and qtg % 2 == 1:
                    # drain: no exps left, scalar engine shares evacuation
                    nc.scalar.copy(ob[:], op[:])
                else:
                    nc.vector.tensor_copy(ob[:], op[:])
                nc.sync.dma_start(outp[128 * qtg : 128 * qtg + 128, :], ob[:])

            def norm_stage_a(st):
                # 1/sum = exp(-ln(sum)); both heads' sums were DMA-staged
                # onto partitions {0,1} of one tile
                q0, zsU, rsin, rs2 = st
                lnS = small.tile([2, 512], f32, tag="lnS")
                nc.scalar.activation(lnS[:], rsin[:], AF.Ln, scale=1.0)
                nc.scalar.activation(rs2[:], lnS[:], AF.Exp, scale=-1.0)

            def norm_stage_b(st):
                # broadcast both reciprocals to 128 partitions with one K=2
                # matmul against the 0/1 selector ee_sb, then normalize the
                # staged z into zT
                q0, zsU, rsin, rs2 = st
                rbP = psum.tile([128, 512], f32, tag="sc", bufs=3, name="rbP")
                nc.tensor.matmul(rbP[:], ee_sb[:], rs2[:], start=True, stop=True)
                for h in range(2):
                    nc.vector.tensor_mul(
                        zT[64 * h : 64 * h + 64, q0 : q0 + 512],
                        zsU[h][0:64, :],
                        rbP[64 * h : 64 * h + 64, :],
                    )

            # ---- one attention unit: 512 queries of one head-pair.
            # Ascending key-tile order: the first AV matmul (start=True) is
            # full-width, so later diagonal tiles' restricted sub-ranges
            # accumulate into already-initialized PSUM columns.
            def emit_unit(qb, hp, last=False):
                nkt = 4 * (qb + 1)
                q0 = SC * hp + 512 * qb
                zp = [
                    psum.tile([65, 512], f32, tag="z", bufs=2, name=f"zp{h}")
                    for h in range(2)
                ]
                # key tiles processed in pairs — [sc,sc,exp,exp,av,av] — so
                # the PE sees only two scores<->AV tile-config switches per
                # pair (LDWEIGHTS only hides behind same-config matmuls)
                for ki in range(0, nkt, 2):
                    pair = (ki, ki + 1)
                    ats = {}
                    for kt in pair:
                        j = kt - 4 * qb  # diagonal index, >= 0 on diagonals
                        off = 128 * j if j >= 0 else 0
                        # scores full-width: the two heads' K=64 matmuls
                        # row-tile into opposite array halves and run
                        # concurrently, so restricting the width would not
                        # save PE time anyway
                        sp = psum.tile(
                            [128, 1024], f32, tag="sc", bufs=3, name="sp"
                        )
                        for h in range(2):
                            nc.tensor.matmul(
                                sp[:, 512 * h : 512 * h + 512],
                                kT[
                                    64 * h : 64 * h + 64,
                                    SC * hp + 128 * kt : SC * hp + 128 * kt + 128,
                                ],
                                qT[64 * h : 64 * h + 64, q0 : q0 + 512],
                                start=True,
                                stop=True,
                            )
                        ats[kt] = (sp, off)
                    for kt in pair:
                        sp, off = ats[kt]
                        at = attnp.tile([128, 1024], bf16)
                        nc.scalar.activation(
                            at[:, off:1024], sp[:, off:1024], AF.Exp, scale=SCALE
                        )
                        if off or kt == 4 * qb:
                            # causal mask reduces to a [128,128] triangle on
                            # the q-subrange [off, off+128) of each head
                            for h in range(2):
                                c = 512 * h + off
                                nc.vector.tensor_mul(
                                    at[:, c : c + 128],
                                    at[:, c : c + 128],
                                    tri_sb[:],
                                )
                        ats[kt] = (at, off)
                    for kt in pair:
                        at, off = ats[kt]
                        for h in range(2):
                            nc.tensor.matmul(
                                zp[h][:, off:512],
                                v_sb[2 * hp + h][:, kt, 0:65],
                                at[:, 512 * h + off : 512 * h + 512],
                                start=(kt == 0),
                                stop=(kt == nkt - 1),
                            )
                    b2 = ki // 2
                    if b2 == 0 or b2 == 2:
                        pump_norm()
                    elif not last:
                        pump_op()
                # evacuate z + sums row to SBUF so PSUM banks free up; a
                # small DMA gathers the two sums rows onto partitions {0,1}
                zsU = [
                    small.tile([65, 512], bf16, tag=f"zsU{h}", name=f"zsU{h}")
                    for h in range(2)
                ]
                rsin = small.tile([2, 512], bf16, tag="rsin")
                rs2 = small.tile([2, 512], bf16, tag="rs2")
                for h in range(2):
                    nc.vector.tensor_copy(zsU[h][:], zp[h][:])
                    # gather the sums rows via the idle GpSimd (SWDGE) queue
                    # so they never wait behind sync-queue transposes
                    nc.gpsimd.dma_start(rsin[h : h + 1, :], zsU[h][64:65, :])
                st = (q0, zsU, rsin, rs2)
                normq.append(lambda st=st: norm_stage_a(st))

                def _stage_b(st=st, qb=qb, hp=hp):
                    norm_stage_b(st)
                    normb_done.add((qb, hp))

                normq.append(_stage_b)
                if hp == 1 and qb < NQB - 1:
                    for qx in range(4):
                        opq.append(
                            ((qb, 1), lambda qtg=4 * qb + qx: emit_outproj_qt(qtg))
                        )
                if hp == 0 and qb == NQB - 1:
                    # last query block: run the hp0 half of its output
                    # projection during the final unit, staged to SBUF fp32;
                    # the tail only needs the hp1 half plus an add
                    for qx in range(4):
                        opq.append(
                            ((qb, 0), lambda qtg=4 * qb + qx: emit_outproj_h0(qtg))
                        )

            # ---- master schedule: QKV blocks interleaved with attention
            # units as dependencies allow, so the PE-dense QKV stream and
            # the ACT-dense softmax stream overlap.
            emit_qkv(0, 0, half_major=True)
            emit_qkv(0, 1)
            emit_unit(0, 0)
            emit_qkv(1, 0)
            emit_unit(0, 1)
            emit_unit(1, 0)
            emit_qkv(1, 1)
            emit_unit(1, 1)
            emit_unit(2, 0)
            emit_unit(2, 1)
            emit_unit(3, 0)
            emit_unit(3, 1, last=True)

            # tail: the held-back hp0 halves of the last query block's
            # output projection fill the PE while the final norm chain
            # (zsU -> sums gather -> ln -> exp) completes; then the norms,
            # then the hp1 halves + adds
            while opq:
                opq.popleft()[1]()
            while normq:
                normq.popleft()()
            for qx in range(4):
                emit_outproj_h1_tail(12 + qx, qx)

    return nc


@functools.lru_cache(maxsize=1)
def _get_nc():
    _install_waitfix()
    return _build_nc()


def _to_bf16(a):
    return np.ascontiguousarray(np.asarray(a, dtype=np.float32)).astype(
        ml_dtypes.bfloat16
    )


def _prepare_in_maps(
    normalized_resid_pre, W_Q, W_K, W_V, W_O, b_Q, b_K, b_V, b_O
):
    x = np.asarray(normalized_resid_pre, dtype=np.float32)
    W_Q = np.asarray(W_Q, dtype=np.float32)
    W_K = np.asarray(W_K, dtype=np.float32)
    W_V = np.asarray(W_V, dtype=np.float32)
    W_O = np.asarray(W_O, dtype=np.float32)
    b_Q = np.asarray(b_Q, dtype=np.float32)
    b_K = np.asarray(b_K, dtype=np.float32)
    b_V = np.asarray(b_V, dtype=np.float32)
    b_O = np.asarray(b_O, dtype=np.float32)

    xT_b = [_to_bf16(x[b].T) for b in range(B)]

    kk = np.arange(128)[:, None]
    qq = np.arange(128)[None, :]
    tri_np = (kk <= qq).astype(np.float32).astype(ml_dtypes.bfloat16)

    ones_np = np.zeros((2, 128), np.float32)
    ones_np[0, :64] = 1.0
    ones_np[1, 64:] = 1.0
    ones_np = ones_np.astype(ml_dtypes.bfloat16)

    in_maps = []
    for c in range(NCORES):
        b = c // 4
        h0 = 4 * (c % 4)
        cols = []
        bcols = []
        for hp in range(NHP):
            ha, hb = h0 + 2 * hp, h0 + 2 * hp + 1
            cols += [W_Q[ha], W_Q[hb], W_K[ha], W_K[hb], W_V[ha], W_V[hb]]
            bcols += [
                np.concatenate([b_Q[ha], b_Q[hb]]),
                np.concatenate([b_K[ha], b_K[hb]]),
                np.concatenate([b_V[ha], b_V[hb]]),
            ]
        wqkv_c = np.concatenate(cols, axis=1)  # [1024, 768]
        # pre-swizzle to the SBUF-linear layout [p, hp, g, a, 128] so the
        # weight loads are contiguous (V slice first for fast startup)
        wqkv_c = (
            wqkv_c.reshape(NDC, 128, NHP, 3, 128)
            .transpose(1, 2, 3, 0, 4)
            .reshape(128, NHP * NDC * 384)
        )
        bqkv_c = np.stack(bcols, axis=1).astype(np.float32)  # [128, 6]
        wo_c = np.concatenate(
            [
                np.concatenate([W_O[h0], W_O[h0 + 1]], axis=0),
                np.concatenate([W_O[h0 + 2], W_O[h0 + 3]], axis=0),
            ],
            axis=1,
        )  # [128, 2048]
        in_maps.append(
            {
                "xT": xT_b[b],
                "wqkv": _to_bf16(wqkv_c),
                "bqkv": np.ascontiguousarray(bqkv_c),
                "wo": _to_bf16(wo_c),
                "tri": tri_np,
                "ones1": ones_np,
            }
        )
    return in_maps, b_O


def _gather(res, b_O):
    out = np.zeros((B, SC, D), np.float32)
    for c, r in enumerate(res.results):
        out[c // 4] += r["outp"].astype(np.float32)
    out += b_O[None, None, :]
    return out


def kernel(
    normalized_resid_pre, W_Q, W_K, W_V, W_O, b_Q, b_K, b_V, b_O, **_unused
):
    in_maps, b_O = _prepare_in_maps(
        normalized_resid_pre, W_Q, W_K, W_V, W_O, b_Q, b_K, b_V, b_O
    )
    nc = _get_nc()
    res = run_bass_kernel_spmd(nc, in_maps, core_ids=list(range(NCORES)))
    return _gather(res, b_O)


def _try_install_profhook():
    """Register the axon NTFF profile hook (the container's antenv stub
    lacks axon_hooks); harmless no-op if anything is missing."""
    try:
        import sys
        import types

        if "antenv.axon_hooks" not in sys.modules:
            mod = types.ModuleType("antenv.axon_hooks")
            hook = [None]
            mod.set_axon_ntff_profile_hook = lambda h: hook.__setitem__(0, h)
            mod.get_axon_ntff_profile_hook = lambda: hook[0]
            sys.modules["antenv.axon_hooks"] = mod
            import antenv

            antenv.axon_hooks = mod
            from trn_agent_boot.trn_boot import _ntff_profile_via_ctypes

            mod.set_axon_ntff_profile_hook(
                _ntff_profile_via_ctypes("/opt/axon/libaxon_pjrt.so")
            )
            import concourse.bass_utils as bu

            bu.upload_artifacts = lambda tmpdir: f"file://{tmpdir}"
    except Exception:
        pass


def kernel_profiled(**inputs):
    """Like kernel() but with NTFF tracing; returns (out, BassKernelResults)."""
    _try_install_profhook()
    inputs = {k: v for k, v in inputs.items()}
    in_maps, b_O = _prepare_in_maps(
        inputs["normalized_resid_pre"],
        inputs["W_Q"],
        inputs["W_K"],
        inputs["W_V"],
        inputs["W_O"],
        inputs["b_Q"],
        inputs["b_K"],
        inputs["b_V"],
        inputs["b_O"],
    )
    nc = _get_nc()
    res = run_bass_kernel_spmd(
        nc, in_maps, core_ids=list(range(NCORES)), trace=True
    )
    return _gather(res, b_O), res


if __name__ == "__main__":
    rng = np.random.default_rng(0)
    inputs = {
        "normalized_resid_pre": rng.standard_normal((B, S, D)).astype(np.float32),
        "W_Q": (rng.standard_normal((NHEAD, D, HDIM)) * 0.02).astype(np.float32),
        "W_K": (rng.standard_normal((NHEAD, D, HDIM)) * 0.02).astype(np.float32),
        "W_V": (rng.standard_normal((NHEAD, D, HDIM)) * 0.02).astype(np.float32),
        "W_O": (rng.standard_normal((NHEAD, HDIM, D)) * 0.02).astype(np.float32),
        "b_Q": np.zeros((NHEAD, HDIM), np.float32),
        "b_K": np.zeros((NHEAD, HDIM), np.float32),
        "b_V": np.zeros((NHEAD, HDIM), np.float32),
        "b_O": np.zeros((D,), np.float32),
    }
    out = kernel(**inputs)
    print("out", out.shape, out.dtype, float(np.abs(out).max()))


# revision 20
# speedup vs baseline: 1.0430x; 1.0430x over previous
"""Causal multi-head attention (B=2, S=2048, D=1024, 16 heads x 64) on 8
Trainium2 NeuronCores.

Sharding: batch x head-group — each core owns one batch (b = core//4) and 4
heads (4*(core%4)..+3), handled as two head-pairs hp in {0,1}. Each core gets
its batch's activations (pre-transposed, bf16) and its 4 heads' weights,
computes q/k/v, causal flash-style attention, and a partial output
projection; the host sums 4 partials per batch and adds b_O.

Device algorithm per core (all matmuls bf16 with fp32 PSUM accumulate):
  - QKV per (pp, hp) block: {q,k,v}T[128 = 2*64 headdims, tok] = W.T @ xT over
    8 contraction chunks. q/k bias+cast on DVE (tensor_scalar_add), v on ACT.
  - V re-laid-out to [pos, headdim] tiles via DMA xbar transposes (frees DVE),
    with a ones column appended so the attention-value matmul also produces
    the softmax denominator for free.
  - scores transposed (key pos on partitions); the two heads' K=64 matmuls
    row-tile into opposite halves of the PE array and run concurrently.
  - causal diagonal restriction: for diagonal key-tiles only the valid query
    sub-range is computed/exp'd/accumulated, and masking reduces to one
    [128,128] triangle multiply per head.
  - exp on ACT straight out of PSUM (scores are O(1); no max subtraction).
  - 1/sum via exp(-ln(sum)) on ACT, broadcast across partitions with a K=2
    matmul, applied while evacuating z.
  - out projection contracts both head-pairs (2x128 dims) into one PSUM tile.
  - QKV blocks and attention units are emission-interleaved so the PE-bound
    QKV stream and the ACT-bound softmax stream overlap.
"""

import functools
from collections import deque

import numpy as np
import ml_dtypes

import concourse.bass as bass
import concourse.tile as tile
import concourse.mybir as mybir
from concourse.bass_utils import run_bass_kernel_spmd

# ---------------------------------------------------------------- wait fix
# This container's walrus accepts at most ONE sync-wait per instruction
# (two for EventSemaphore); Tile emits several. Hoist the excess onto NoOps
# inserted just before the over-subscribed instruction on the same engine.
import json as _json

_WAIT_CAP = {"EventSemaphore": 2}


def _split_waits(doc):
    n = [0]

    def fix_block(block):
        insts = block.get("instructions")
        if not isinstance(insts, list):
            return
        out = []
        for inst in insts:
            si = inst.get("sync_info")
            waits = si.get("on_wait") if si else None
            cap = _WAIT_CAP.get(inst.get("opcode"), 1)
            if waits and len(waits) > cap:
                for w in waits[cap:]:
                    n[0] += 1
                    out.append(
                        {
                            "name": f"WSPL-{n[0]}",
                            "opcode": "NoOp",
                            "engine": inst["engine"],
                            "ins": [],
                            "outs": [],
                            "sync_info": {"on_wait": [w], "on_update": []},
                        }
                    )
                si["on_wait"] = waits[:cap]
            out.append(inst)
        block["instructions"] = out

    def walk(o):
        if isinstance(o, dict):
            if "instructions" in o:
                fix_block(o)
            for v in o.values():
                walk(v)
        elif isinstance(o, list):
            for v in o:
                walk(v)

    walk(doc)
    return doc


_waitfix_done = False


def _install_waitfix():
    global _waitfix_done
    if _waitfix_done:
        return
    _waitfix_done = True
    orig = bass.Bass.to_json_bytes

    def to_json_bytes(self, *a, **kw):
        doc = _json.loads(orig(self, *a, **kw))
        return _json.dumps(_split_waits(doc)).encode()

    bass.Bass.to_json_bytes = to_json_bytes


# ---------------------------------------------------------------- constants
B, S, D = 2, 2048, 1024
NHEAD, HDIM = 16, 64
NCORES = 8
SC = S  # tokens per core (one batch)
NHP = 2  # head-pairs per core (4 heads)
SCALE = 1.0 / 8.0  # 1/sqrt(HDIM)

bf16 = mybir.dt.bfloat16
f32 = mybir.dt.float32
AF = mybir.ActivationFunctionType

NDC = D // 128  # 8 contraction chunks
NKT = SC // 128  # 16 key tiles
NQB = SC // 512  # 4 query blocks
VT = NHP * SC  # 4096 virtual token-cols (hp-major)


def _build_nc():
    nc = bass.Bass()
    xT = nc.dram_tensor("xT", [D, SC], bf16, kind="ExternalInput")
    wqkv = nc.dram_tensor("wqkv", [128, NHP * NDC * 384], bf16, kind="ExternalInput")
    bqkv = nc.dram_tensor("bqkv", [128, NHP * 3], f32, kind="ExternalInput")
    wo = nc.dram_tensor("wo", [128, NHP * D], bf16, kind="ExternalInput")
    tri = nc.dram_tensor("tri", [128, 128], bf16, kind="ExternalInput")
    ones1 = nc.dram_tensor("ones1", [2, 128], bf16, kind="ExternalInput")
    outp = nc.dram_tensor("outp", [SC, D], bf16, kind="ExternalOutput")

    with tile.TileContext(nc) as tc:
        with (
            tc.tile_pool(name="const", bufs=1) as const,
            tc.tile_pool(name="attn", bufs=8) as attnp,
            tc.tile_pool(name="obuf", bufs=4) as obufp,
            tc.tile_pool(name="small", bufs=4) as small,
            tc.tile_pool(name="psum", bufs=2, space="PSUM") as psum,
        ):
            # ---- constant tiles
            w_sb = const.tile([128, NHP, 3, NDC, 128], bf16)
            xt_sb = const.tile([128, NDC, SC], bf16)
            bias_sb = const.tile([128, NHP * 3], f32)
            wo_sb = const.tile([128, NHP, D], bf16)
            tri_sb = const.tile([128, 128], bf16)
            ee_sb = const.tile([2, 128], bf16)

            qT = const.tile([128, VT], bf16)
            kT = const.tile([128, VT], bf16)
            vT = const.tile([128, VT], bf16)
            zT = const.tile([128, VT], bf16)

            # v in [pos, headdim] layout, one per head; col 64 = ones so the
            # AV matmul produces softmax denominators; padded to 80 cols for
            # the 32B-aligned DMA-transpose destination stride.
            v_sb = []
            for i in range(4):
                v = const.tile([128, NKT, 80], bf16, name=f"v_sb{i}")
                nc.gpsimd.memset(v[:, :, 64], 1.0)
                v_sb.append(v)

            wq_r = wqkv[:].rearrange("p (h g a c) -> p h g a c", h=NHP, g=3, a=NDC)
            xTr = xT[:].rearrange("(a p) m -> p a m", p=128)

            # ---- DMA order = first-use order (hp0's weights contiguous,
            # then the x chunks the first QKV block reads, etc.)
            nc.sync.dma_start(w_sb[:, 0, 2], wq_r[:, 0, 2])
            nc.sync.dma_start(xt_sb[:, 0:2, 0:512], xTr[:, 0:2, 0:512])
            nc.sync.dma_start(xt_sb[:, 2:4, 0:512], xTr[:, 2:4, 0:512])
            nc.sync.dma_start(xt_sb[:, 4:6, 0:512], xTr[:, 4:6, 0:512])
            nc.sync.dma_start(xt_sb[:, 6:8, 0:512], xTr[:, 6:8, 0:512])
            nc.sync.dma_start(xt_sb[:, :, 512:1024], xTr[:, :, 512:1024])
            nc.sync.dma_start(w_sb[:, 0, 0:2], wq_r[:, 0, 0:2])
            nc.sync.dma_start(bias_sb[:], bqkv[:])
            nc.sync.dma_start(w_sb[:, 1], wq_r[:, 1])
            nc.sync.dma_start(ee_sb[:], ones1[:])
            nc.sync.dma_start(tri_sb[:], tri[:])
            nc.sync.dma_start(xt_sb[:, :, 1024:1536], xTr[:, :, 1024:1536])
            nc.sync.dma_start(xt_sb[:, :, 1536:2048], xTr[:, :, 1536:2048])
            wo_r = wo[:].rearrange("p (h d) -> p h d", h=NHP)
            nc.sync.dma_start(wo_sb[:], wo_r)

            # ---- deferred-work queues: each unit's norm stages go on a
            # high-priority queue pumped early in the next emission block
            # (stage b late enough that its PE matmul never stalls on the
            # ln/exp chain); output projections pump from a background queue
            # gated on their query block's norms having been emitted.
            normq = deque()
            opq = deque()
            normb_done = set()

            def pump_norm():
                if normq:
                    normq.popleft()()

            def pump_op():
                if opq and opq[0][0] in normb_done:
                    opq.popleft()[1]()

            # ---- QKV projection for one (pp, hp): 1024 tokens, one
            # head-pair; V group first so its re-layout transposes start
            # early.
            def emit_qkv(pp, hp, half_major=False):
                gs = (2, 0, 1)  # V, Q, K
                pst = {
                    g: psum.tile(
                        [128, 1024], f32, tag="sc", bufs=2, name=f"qkv_ps{g}"
                    )
                    for g in gs
                }

                def mms(g, half):
                    pt = 2 * pp + half
                    dst = pst[g][:, 512 * half : 512 * half + 512]
                    for di in range(NDC):
                        nc.tensor.matmul(
                            dst,
                            w_sb[:, hp, g, di, :],
                            xt_sb[:, di, 512 * pt : 512 * pt + 512],
                            start=(di == 0),
                            stop=(di == NDC - 1),
                        )

                if half_major:
                    # first block: run all groups on the first 512-token slab
                    # so compute starts before the second slab's DMA lands
                    for g in gs:
                        mms(g, 0)
                for g in gs:
                    if half_major:
                        mms(g, 1)
                    else:
                        mms(g, 0)
                        mms(g, 1)
                    ps = pst[g]
                    dc = SC * hp + 1024 * pp
                    bcol = 3 * hp + g
                    if g == 2:
                        # v: bias+cast on ACT, then xbar-transpose each
                        # 128-token tile into [pos, headdim] layout
                        nc.scalar.activation(
                            vT[:, dc : dc + 1024],
                            ps[:],
                            AF.Identity,
                            bias=bias_sb[:, bcol : bcol + 1],
                            scale=1.0,
                        )
                        # one batched xbar transpose per head: [64, 1024] ->
                        # [128 pos, 8 tiles, 64] (row r of the transpose maps
                        # to (tile r//128, partition r%128))
                        for h in range(2):
                            nc.sync.dma_start_transpose(
                                v_sb[2 * hp + h][:, 8 * pp : 8 * pp + 8, 0:64],
                                vT[64 * h : 64 * h + 64, dc : dc + 1024],
                            )
                    else:
                        tgt = qT if g == 0 else kT
                        nc.vector.tensor_scalar_add(
                            tgt[:, dc : dc + 1024],
                            ps[:],
                            bias_sb[:, bcol : bcol + 1],
                        )
                    if g == 0:
                        pump_op()
                    else:
                        pump_norm()

            # ---- output projection for one 128-token tile; contracts both
            # head-pairs' z into one PSUM accumulation.
            def emit_outproj_qt(qtg, tail=False):
                # mid-kernel outprojs use a dedicated PSUM slot so they never
                # evict the scores rotation; tail ones reuse the (by then
                # idle) scores slots for 2-deep evac overlap
                op = psum.tile(
                    [128, 1024],
                    f32,
                    tag="sc" if tail else "op",
                    bufs=2 if tail else 1,
                    name="op",
                )
                for dh in range(2):
                    for hp in range(NHP):
                        nc.tensor.matmul(
                            op[:, 512 * dh : 512 * dh + 512],
                            zT[:, SC * hp + 128 * qtg : SC * hp + 128 * qtg + 128],
                            wo_sb[:, hp, 512 * dh : 512 * dh + 512],
                            start=(hp == 0),
                            stop=(hp == NHP - 1),
                        )
                ob = obufp.tile([128, 1024], bf16, name="ob")
                if tail and qtg % 2 == 1:
                    # drain: no exps left, scalar engine shares evacuation
                    nc.scalar.copy(ob[:], op[:])
                else:
                    nc.vector.tensor_copy(ob[:], op[:])
                nc.sync.dma_start(outp[128 * qtg : 128 * qtg + 128, :], ob[:])

            def norm_stage_a(st):
                # 1/sum = exp(-ln(sum)); both heads' sums were DMA-staged
                # onto partitions {0,1} of one tile
                q0, zsU, rsin, rs2 = st
                lnS = small.tile([2, 512], f32, tag="lnS")
                nc.scalar.activation(lnS[:], rsin[:], AF.Ln, scale=1.0)
                nc.scalar.activation(rs2[:], lnS[:], AF.Exp, scale=-1.0)

            def norm_stage_b(st):
                # broadcast both reciprocals to 128 partitions with one K=2
                # matmul against the 0/1 selector ee_sb, then normalize the
                # staged z into zT
                q0, zsU, rsin, rs2 = st
                rbP = psum.tile([128, 512], f32, tag="op", bufs=1, name="rbP")
                nc.tensor.matmul(rbP[:], ee_sb[:], rs2[:], start=True, stop=True)
                for h in range(2):
                    nc.vector.tensor_mul(
                        zT[64 * h : 64 * h + 64, q0 : q0 + 512],
                        zsU[h][0:64, :],
                        rbP[64 * h : 64 * h + 64, :],
                    )

            # ---- one attention unit: 512 queries of one head-pair.
            # Ascending key-tile order: the first AV matmul (start=True) is
            # full-width, so later diagonal tiles' restricted sub-ranges
            # accumulate into already-initialized PSUM columns.
            def emit_unit(qb, hp, last=False):
                nkt = 4 * (qb + 1)
                q0 = SC * hp + 512 * qb
                zp = [
                    psum.tile([65, 512], f32, tag="z", bufs=2, name=f"zp{h}")
                    for h in range(2)
                ]
                for kt in range(nkt):
                    j = kt - 4 * qb  # diagonal index, >= 0 on diagonal tiles
                    off = 128 * j if j >= 0 else 0
                    # scores full-width: the two heads' K=64 matmuls row-tile
                    # into opposite array halves and run concurrently, so a
                    # restricted width would not save PE time anyway
                    sp = psum.tile([128, 1024], f32, tag="sc", bufs=2, name="sp")
                    for h in range(2):
                        nc.tensor.matmul(
                            sp[:, 512 * h : 512 * h + 512],
                            kT[
                                64 * h : 64 * h + 64,
                                SC * hp + 128 * kt : SC * hp + 128 * kt + 128,
                            ],
                            qT[64 * h : 64 * h + 64, q0 : q0 + 512],
                            start=True,
                            stop=True,
                        )
                    at = attnp.tile([128, 1024], bf16)
                    nc.scalar.activation(
                        at[:, off:1024], sp[:, off:1024], AF.Exp, scale=SCALE
                    )
                    if j >= 0:
                        # causal mask reduces to a [128,128] triangle on the
                        # q-subrange [off, off+128) of each head
                        for h in range(2):
                            c = 512 * h + off
                            nc.vector.tensor_mul(
                                at[:, c : c + 128], at[:, c : c + 128], tri_sb[:]
                            )
                    for h in range(2):
                        nc.tensor.matmul(
                            zp[h][:, off:512],
                            v_sb[2 * hp + h][:, kt, 0:65],
                            at[:, 512 * h + off : 512 * h + 512],
                            start=(kt == 0),
                            stop=(kt == nkt - 1),
                        )
                    if kt == 1 or kt == 5:
                        pump_norm()
                    elif kt % 2 == 1 and not last:
                        pump_op()
                # evacuate z + sums row to SBUF so PSUM banks free up; a
                # small DMA gathers the two sums rows onto partitions {0,1}
                zsU = [
                    small.tile([65, 512], bf16, tag=f"zsU{h}", name=f"zsU{h}")
                    for h in range(2)
                ]
                rsin = small.tile([2, 512], bf16, tag="rsin")
                rs2 = small.tile([2, 512], bf16, tag="rs2")
                for h in range(2):
                    nc.vector.tensor_copy(zsU[h][:], zp[h][:])
                    # gather the sums rows via the idle GpSimd (SWDGE) queue
                    # so they never wait behind sync-queue transposes; the
                    # last unit uses the (by then idle) low-latency HWDGE
                    eng = nc.sync if last else nc.gpsimd
                    eng.dma_start(rsin[h : h + 1, :], zsU[h][64:65, :])
                st = (q0, zsU, rsin, rs2)
                normq.append(lambda st=st: norm_stage_a(st))

                def _stage_b(st=st, qb=qb, hp=hp):
                    norm_stage_b(st)
                    normb_done.add((qb, hp))

                normq.append(_stage_b)
                if hp == 1 and qb < NQB - 1:
                    for qx in range(4):
                        opq.append(
                            ((qb, 1), lambda qtg=4 * qb + qx: emit_outproj_qt(qtg))
                        )


            # ---- master schedule: QKV blocks interleaved with attention
            # units as dependencies allow, so the PE-dense QKV stream and
            # the ACT-dense softmax stream overlap.
            emit_qkv(0, 0, half_major=True)
            emit_qkv(0, 1)
            emit_unit(0, 0)
            emit_qkv(1, 0)
            emit_unit(0, 1)
            emit_unit(1, 0)
            emit_qkv(1, 1)
            emit_unit(1, 1)
            emit_unit(2, 0)
            emit_unit(2, 1)
            emit_unit(3, 0)
            emit_unit(3, 1, last=True)

            # drain the tail: last norms, then the last query block's
            # output projections with evac split across ACT and DVE
            while opq:
                opq.popleft()[1]()
            while normq:
                normq.popleft()()
            for qx in range(4):
                emit_outproj_qt(12 + qx, tail=True)

    return nc


@functools.lru_cache(maxsize=1)
def _get_nc():
    _install_waitfix()
    return _build_nc()


def _to_bf16(a):
    return np.ascontiguousarray(np.asarray(a, dtype=np.float32)).astype(
        ml_dtypes.bfloat16
    )


def _prepare_in_maps(
    normalized_resid_pre, W_Q, W_K, W_V, W_O, b_Q, b_K, b_V, b_O
):
    x = np.asarray(normalized_resid_pre, dtype=np.float32)
    W_Q = np.asarray(W_Q, dtype=np.float32)
    W_K = np.asarray(W_K, dtype=np.float32)
    W_V = np.asarray(W_V, dtype=np.float32)
    W_O = np.asarray(W_O, dtype=np.float32)
    b_Q = np.asarray(b_Q, dtype=np.float32)
    b_K = np.asarray(b_K, dtype=np.float32)
    b_V = np.asarray(b_V, dtype=np.float32)
    b_O = np.asarray(b_O, dtype=np.float32)

    xT_b = [_to_bf16(x[b].T) for b in range(B)]

    kk = np.arange(128)[:, None]
    qq = np.arange(128)[None, :]
    tri_np = (kk <= qq).astype(np.float32).astype(ml_dtypes.bfloat16)

    ones_np = np.zeros((2, 128), np.float32)
    ones_np[0, :64] = 1.0
    ones_np[1, 64:] = 1.0
    ones_np = ones_np.astype(ml_dtypes.bfloat16)

    in_maps = []
    for c in range(NCORES):
        b = c // 4
        h0 = 4 * (c % 4)
        cols = []
        bcols = []
        for hp in range(NHP):
            ha, hb = h0 + 2 * hp, h0 + 2 * hp + 1
            cols += [W_Q[ha], W_Q[hb], W_K[ha], W_K[hb], W_V[ha], W_V[hb]]
            bcols += [
                np.concatenate([b_Q[ha], b_Q[hb]]),
                np.concatenate([b_K[ha], b_K[hb]]),
                np.concatenate([b_V[ha], b_V[hb]]),
            ]
        wqkv_c = np.concatenate(cols, axis=1)  # [1024, 768]
        # pre-swizzle to the SBUF-linear layout [p, hp, g, a, 128] so the
        # weight loads are contiguous (V slice first for fast startup)
        wqkv_c = (
            wqkv_c.reshape(NDC, 128, NHP, 3, 128)
            .transpose(1, 2, 3, 0, 4)
            .reshape(128, NHP * NDC * 384)
        )
        bqkv_c = np.stack(bcols, axis=1).astype(np.float32)  # [128, 6]
        wo_c = np.concatenate(
            [
                np.concatenate([W_O[h0], W_O[h0 + 1]], axis=0),
                np.concatenate([W_O[h0 + 2], W_O[h0 + 3]], axis=0),
            ],
            axis=1,
        )  # [128, 2048]
        in_maps.append(
            {
                "xT": xT_b[b],
                "wqkv": _to_bf16(wqkv_c),
                "bqkv": np.ascontiguousarray(bqkv_c),
                "wo": _to_bf16(wo_c),
                "tri": tri_np,
                "ones1": ones_np,
            }
        )
    return in_maps, b_O


def _gather(res, b_O):
    out = np.zeros((B, SC, D), np.float32)
    for c, r in enumerate(res.results):
        out[c // 4] += r["outp"].astype(np.float32)
    out += b_O[None, None, :]
    return out


def kernel(
    normalized_resid_pre, W_Q, W_K, W_V, W_O, b_Q, b_K, b_V, b_O, **_unused
):
    in_maps, b_O = _prepare_in_maps(
        normalized_resid_pre, W_Q, W_K, W_V, W_O, b_Q, b_K, b_V, b_O
    )
    nc = _get_nc()
    res = run_bass_kernel_spmd(nc, in_maps, core_ids=list(range(NCORES)))
    return _gather(res, b_O)


def _try_install_profhook():
    """Register the axon NTFF profile hook (the container's antenv stub
    lacks axon_hooks); harmless no-op if anything is missing."""
    try:
        import sys
        import types

        if "antenv.axon_hooks" not in sys.modules:
            mod = types.ModuleType("antenv.axon_hooks")
            hook = [None]
            mod.set_axon_ntff_profile_hook = lambda h: hook.__setitem__(0, h)
            mod.get_axon_ntff_profile_hook = lambda: hook[0]
            sys.modules["antenv.axon_hooks"] = mod
            import antenv

            antenv.axon_hooks = mod
            from trn_agent_boot.trn_boot import _ntff_profile_via_ctypes

            mod.set_axon_ntff_profile_hook(
                _ntff_profile_via_ctypes("/opt/axon/libaxon_pjrt.so")
            )
            import concourse.bass_utils as bu

            bu.upload_artifacts = lambda tmpdir: f"file://{tmpdir}"
    except Exception:
        pass


def kernel_profiled(**inputs):
    """Like kernel() but with NTFF tracing; returns (out, BassKernelResults)."""
    _try_install_profhook()
    inputs = {k: v for k, v in inputs.items()}
    in_maps, b_O = _prepare_in_maps(
        inputs["normalized_resid_pre"],
        inputs["W_Q"],
        inputs["W_K"],
        inputs["W_V"],
        inputs["W_O"],
        inputs["b_Q"],
        inputs["b_K"],
        inputs["b_V"],
        inputs["b_O"],
    )
    nc = _get_nc()
    res = run_bass_kernel_spmd(
        nc, in_maps, core_ids=list(range(NCORES)), trace=True
    )
    return _gather(res, b_O), res


if __name__ == "__main__":
    rng = np.random.default_rng(0)
    inputs = {
        "normalized_resid_pre": rng.standard_normal((B, S, D)).astype(np.float32),
        "W_Q": (rng.standard_normal((NHEAD, D, HDIM)) * 0.02).astype(np.float32),
        "W_K": (rng.standard_normal((NHEAD, D, HDIM)) * 0.02).astype(np.float32),
        "W_V": (rng.standard_normal((NHEAD, D, HDIM)) * 0.02).astype(np.float32),
        "W_O": (rng.standard_normal((NHEAD, HDIM, D)) * 0.02).astype(np.float32),
        "b_Q": np.zeros((NHEAD, HDIM), np.float32),
        "b_K": np.zeros((NHEAD, HDIM), np.float32),
        "b_V": np.zeros((NHEAD, HDIM), np.float32),
        "b_O": np.zeros((D,), np.float32),
    }
    out = kernel(**inputs)
    print("out", out.shape, out.dtype, float(np.abs(out).max()))


# revision 25
# speedup vs baseline: 1.0454x; 1.0024x over previous
"""Causal multi-head attention (B=2, S=2048, D=1024, 16 heads x 64) on 8
Trainium2 NeuronCores.

Sharding: batch x head-group — each core owns one batch (b = core//4) and 4
heads (4*(core%4)..+3), handled as two head-pairs hp in {0,1}. Each core gets
its batch's activations (pre-transposed, bf16) and its 4 heads' weights,
computes q/k/v, causal flash-style attention, and a partial output
projection; the host sums 4 partials per batch and adds b_O.

Device algorithm per core (all matmuls bf16 with fp32 PSUM accumulate):
  - QKV per (pp, hp) block: {q,k,v}T[128 = 2*64 headdims, tok] = W.T @ xT over
    8 contraction chunks. q/k bias+cast on DVE (tensor_scalar_add), v on ACT.
  - V re-laid-out to [pos, headdim] tiles via DMA xbar transposes (frees DVE),
    with a ones column appended so the attention-value matmul also produces
    the softmax denominator for free.
  - scores transposed (key pos on partitions); the two heads' K=64 matmuls
    row-tile into opposite halves of the PE array and run concurrently.
  - causal diagonal restriction: for diagonal key-tiles only the valid query
    sub-range is computed/exp'd/accumulated, and masking reduces to one
    [128,128] triangle multiply per head.
  - exp on ACT straight out of PSUM (scores are O(1); no max subtraction).
  - 1/sum via exp(-ln(sum)) on ACT, broadcast across partitions with a K=2
    matmul, applied while evacuating z.
  - out projection contracts both head-pairs (2x128 dims) into one PSUM tile.
  - QKV blocks and attention units are emission-interleaved so the PE-bound
    QKV stream and the ACT-bound softmax stream overlap.
"""

import functools
from collections import deque

import numpy as np
import ml_dtypes

import concourse.bass as bass
import concourse.tile as tile
import concourse.mybir as mybir
from concourse.bass_utils import run_bass_kernel_spmd

# ---------------------------------------------------------------- wait fix
# This container's walrus accepts at most ONE sync-wait per instruction
# (two for EventSemaphore); Tile emits several. Hoist the excess onto NoOps
# inserted just before the over-subscribed instruction on the same engine.
import json as _json

_WAIT_CAP = {"EventSemaphore": 2}


def _split_waits(doc):
    n = [0]

    def fix_block(block):
        insts = block.get("instructions")
        if not isinstance(insts, list):
            return
        out = []
        for inst in insts:
            si = inst.get("sync_info")
            waits = si.get("on_wait") if si else None
            cap = _WAIT_CAP.get(inst.get("opcode"), 1)
            if waits and len(waits) > cap:
                for w in waits[cap:]:
                    n[0] += 1
                    out.append(
                        {
                            "name": f"WSPL-{n[0]}",
                            "opcode": "NoOp",
                            "engine": inst["engine"],
                            "ins": [],
                            "outs": [],
                            "sync_info": {"on_wait": [w], "on_update": []},
                        }
                    )
                si["on_wait"] = waits[:cap]
            out.append(inst)
        block["instructions"] = out

    def walk(o):
        if isinstance(o, dict):
            if "instructions" in o:
                fix_block(o)
            for v in o.values():
                walk(v)
        elif isinstance(o, list):
            for v in o:
                walk(v)

    walk(doc)
    return doc


_waitfix_done = False


def _install_waitfix():
    global _waitfix_done
    if _waitfix_done:
        return
    _waitfix_done = True
    orig = bass.Bass.to_json_bytes

    def to_json_bytes(self, *a, **kw):
        doc = _json.loads(orig(self, *a, **kw))
        return _json.dumps(_split_waits(doc)).encode()

    bass.Bass.to_json_bytes = to_json_bytes


# ---------------------------------------------------------------- constants
B, S, D = 2, 2048, 1024
NHEAD, HDIM = 16, 64
NCORES = 8
SC = S  # tokens per core (one batch)
NHP = 2  # head-pairs per core (4 heads)
SCALE = 1.0 / 8.0  # 1/sqrt(HDIM)

bf16 = mybir.dt.bfloat16
f32 = mybir.dt.float32
AF = mybir.ActivationFunctionType

NDC = D // 128  # 8 contraction chunks
NKT = SC // 128  # 16 key tiles
NQB = SC // 512  # 4 query blocks
VT = NHP * SC  # 4096 virtual token-cols (hp-major)


def _build_nc():
    nc = bass.Bass()
    xT = nc.dram_tensor("xT", [D, SC], bf16, kind="ExternalInput")
    wqkv = nc.dram_tensor("wqkv", [128, NHP * NDC * 384], bf16, kind="ExternalInput")
    bqkv = nc.dram_tensor("bqkv", [128, NHP * 3], f32, kind="ExternalInput")
    wo = nc.dram_tensor("wo", [128, NHP * D], bf16, kind="ExternalInput")
    tri = nc.dram_tensor("tri", [128, 128], bf16, kind="ExternalInput")
    ones1 = nc.dram_tensor("ones1", [2, 128], bf16, kind="ExternalInput")
    outp = nc.dram_tensor("outp", [SC, D], bf16, kind="ExternalOutput")

    with tile.TileContext(nc) as tc:
        with (
            tc.tile_pool(name="const", bufs=1) as const,
            tc.tile_pool(name="attn", bufs=8) as attnp,
            tc.tile_pool(name="obuf", bufs=4) as obufp,
            tc.tile_pool(name="small", bufs=4) as small,
            tc.tile_pool(name="psum", bufs=2, space="PSUM") as psum,
        ):
            # ---- constant tiles
            w_sb = const.tile([128, NHP, 3, NDC, 128], bf16)
            xt_sb = const.tile([128, NDC, SC], bf16)
            bias_sb = const.tile([128, NHP * 3], f32)
            wo_sb = const.tile([128, NHP, D], bf16)
            tri_sb = const.tile([128, 128], bf16)
            ee_sb = const.tile([2, 128], bf16)

            # q/k/z in per-(head-pair, 1024-token / query-block) tiles:
            # separate tiles keep Tile's dependency tracking from creating
            # false write-after-read orderings between unrelated blocks
            qTt = [
                [const.tile([128, 1024], bf16, name=f"qT{h}_{p}") for p in range(2)]
                for h in range(NHP)
            ]
            kTt = [
                [const.tile([128, 1024], bf16, name=f"kT{h}_{p}") for p in range(2)]
                for h in range(NHP)
            ]
            zTt = [
                [const.tile([128, 512], bf16, name=f"zT{h}_{q}") for q in range(NQB)]
                for h in range(NHP)
            ]
            vT = const.tile([128, VT], bf16)

            # v in [pos, headdim] layout, one per head; col 64 = ones so the
            # AV matmul produces softmax denominators; padded to 80 cols for
            # the 32B-aligned DMA-transpose destination stride.
            v_sb = []
            for i in range(4):
                v = const.tile([128, NKT, 80], bf16, name=f"v_sb{i}")
                nc.gpsimd.memset(v[:, :, 64], 1.0)
                v_sb.append(v)

            wq_r = wqkv[:].rearrange("p (h g a c) -> p h g a c", h=NHP, g=3, a=NDC)
            xTr = xT[:].rearrange("(a p) m -> p a m", p=128)

            # ---- DMA order = first-use order (hp0's weights contiguous,
            # then the x chunks the first QKV block reads, etc.)
            nc.sync.dma_start(w_sb[:, 0, 2], wq_r[:, 0, 2])
            nc.sync.dma_start(xt_sb[:, 0:2, 0:512], xTr[:, 0:2, 0:512])
            nc.sync.dma_start(xt_sb[:, 2:4, 0:512], xTr[:, 2:4, 0:512])
            nc.sync.dma_start(xt_sb[:, 4:6, 0:512], xTr[:, 4:6, 0:512])
            nc.sync.dma_start(xt_sb[:, 6:8, 0:512], xTr[:, 6:8, 0:512])
            nc.sync.dma_start(xt_sb[:, :, 512:1024], xTr[:, :, 512:1024])
            nc.sync.dma_start(w_sb[:, 0, 0:2], wq_r[:, 0, 0:2])
            nc.sync.dma_start(bias_sb[:], bqkv[:])
            nc.sync.dma_start(w_sb[:, 1], wq_r[:, 1])
            nc.sync.dma_start(ee_sb[:], ones1[:])
            nc.sync.dma_start(tri_sb[:], tri[:])
            nc.sync.dma_start(xt_sb[:, :, 1024:1536], xTr[:, :, 1024:1536])
            nc.sync.dma_start(xt_sb[:, :, 1536:2048], xTr[:, :, 1536:2048])
            wo_r = wo[:].rearrange("p (h d) -> p h d", h=NHP)
            nc.sync.dma_start(wo_sb[:], wo_r)

            # ---- deferred-work queues: each unit's norm stages go on a
            # high-priority queue pumped early in the next emission block
            # (stage b late enough that its PE matmul never stalls on the
            # ln/exp chain); output projections pump from a background queue
            # gated on their query block's norms having been emitted.
            normq = deque()
            opq = deque()
            normb_done = set()

            def pump_norm():
                if normq:
                    normq.popleft()()

            def pump_op():
                if opq and opq[0][0] in normb_done:
                    opq.popleft()[1]()

            # ---- QKV projection for one (pp, hp): 1024 tokens, one
            # head-pair; V group first so its re-layout transposes start
            # early.
            def emit_qkv(pp, hp, half_major=False):
                gs = (2, 0, 1)  # V, Q, K
                pst = {
                    g: psum.tile(
                        [128, 1024], f32, tag="sc", bufs=2, name=f"qkv_ps{g}"
                    )
                    for g in gs
                }

                def mms(g, half):
                    pt = 2 * pp + half
                    dst = pst[g][:, 512 * half : 512 * half + 512]
                    for di in range(NDC):
                        nc.tensor.matmul(
                            dst,
                            w_sb[:, hp, g, di, :],
                            xt_sb[:, di, 512 * pt : 512 * pt + 512],
                            start=(di == 0),
                            stop=(di == NDC - 1),
                        )

                if half_major:
                    # first block: run all groups on the first 512-token slab
                    # so compute starts before the second slab's DMA lands
                    for g in gs:
                        mms(g, 0)
                for g in gs:
                    if half_major:
                        mms(g, 1)
                    else:
                        mms(g, 0)
                        mms(g, 1)
                    ps = pst[g]
                    dc = SC * hp + 1024 * pp
                    bcol = 3 * hp + g
                    if g == 2:
                        # v: bias+cast on ACT, then xbar-transpose each
                        # 128-token tile into [pos, headdim] layout
                        nc.scalar.activation(
                            vT[:, dc : dc + 1024],
                            ps[:],
                            AF.Identity,
                            bias=bias_sb[:, bcol : bcol + 1],
                            scale=1.0,
                        )
                        # one batched xbar transpose per head: [64, 1024] ->
                        # [128 pos, 8 tiles, 64] (row r of the transpose maps
                        # to (tile r//128, partition r%128))
                        for h in range(2):
                            nc.sync.dma_start_transpose(
                                v_sb[2 * hp + h][:, 8 * pp : 8 * pp + 8, 0:64],
                                vT[64 * h : 64 * h + 64, dc : dc + 1024],
                            )
                    else:
                        tgt = qTt[hp][pp] if g == 0 else kTt[hp][pp]
                        nc.vector.tensor_scalar_add(
                            tgt[:],
                            ps[:],
                            bias_sb[:, bcol : bcol + 1],
                        )
                    if g == 0:
                        pump_op()
                    else:
                        pump_norm()

            # ---- output projection for one 128-token tile; contracts both
            # head-pairs' z into one PSUM accumulation.
            def emit_outproj_qt(qtg, tail=False):
                # mid-kernel outprojs use a dedicated PSUM slot so they never
                # evict the scores rotation; tail ones reuse the (by then
                # idle) scores slots for 2-deep evac overlap
                op = psum.tile(
                    [128, 1024],
                    f32,
                    tag="sc" if tail else "op",
                    bufs=2 if tail else 1,
                    name="op",
                )
                for dh in range(2):
                    for hp in range(NHP):
                        zt = zTt[hp][qtg // 4]
                        nc.tensor.matmul(
                            op[:, 512 * dh : 512 * dh + 512],
                            zt[:, 128 * (qtg % 4) : 128 * (qtg % 4) + 128],
                            wo_sb[:, hp, 512 * dh : 512 * dh + 512],
                            start=(hp == 0),
                            stop=(hp == NHP - 1),
                        )
                ob = obufp.tile([128, 1024], bf16, name="ob")
                if tail and qtg % 2 == 1:
                    # drain: no exps left, scalar engine shares evacuation
                    nc.scalar.copy(ob[:], op[:])
                else:
                    nc.vector.tensor_copy(ob[:], op[:])
                nc.sync.dma_start(outp[128 * qtg : 128 * qtg + 128, :], ob[:])

            def norm_stage_a(st):
                # 1/sum = exp(-ln(sum)); both heads' sums were DMA-staged
                # onto partitions {0,1} of one tile
                qb, hp, zsU, rsin, rs2 = st
                lnS = small.tile([2, 512], f32, tag="lnS")
                nc.scalar.activation(lnS[:], rsin[:], AF.Ln, scale=1.0)
                nc.scalar.activation(rs2[:], lnS[:], AF.Exp, scale=-1.0)

            def norm_stage_b(st):
                # broadcast both reciprocals to 128 partitions with one K=2
                # matmul against the 0/1 selector ee_sb, then normalize the
                # staged z into its zT tile
                qb, hp, zsU, rsin, rs2 = st
                rbP = psum.tile([128, 512], f32, tag="op", bufs=1, name="rbP")
                nc.tensor.matmul(rbP[:], ee_sb[:], rs2[:], start=True, stop=True)
                for h in range(2):
                    nc.vector.tensor_mul(
                        zTt[hp][qb][64 * h : 64 * h + 64, :],
                        zsU[h][0:64, :],
                        rbP[64 * h : 64 * h + 64, :],
                    )

            # ---- one attention unit: 512 queries of one head-pair.
            # Ascending key-tile order: the first AV matmul (start=True) is
            # full-width, so later diagonal tiles' restricted sub-ranges
            # accumulate into already-initialized PSUM columns.
            def emit_unit(qb, hp, last=False):
                nkt = 4 * (qb + 1)
                zp = [
                    psum.tile([65, 512], f32, tag="z", bufs=2, name=f"zp{h}")
                    for h in range(2)
                ]
                for kt in range(nkt):
                    j = kt - 4 * qb  # diagonal index, >= 0 on diagonal tiles
                    off = 128 * j if j >= 0 else 0
                    # scores full-width: the two heads' K=64 matmuls row-tile
                    # into opposite array halves and run concurrently, so a
                    # restricted width would not save PE time anyway
                    sp = psum.tile([128, 1024], f32, tag="sc", bufs=2, name="sp")
                    kts_ = kTt[hp][kt // 8]
                    qts_ = qTt[hp][qb // 2]
                    qo = 512 * (qb % 2)
                    ko = 128 * (kt % 8)
                    for h in range(2):
                        nc.tensor.matmul(
                            sp[:, 512 * h : 512 * h + 512],
                            kts_[64 * h : 64 * h + 64, ko : ko + 128],
                            qts_[64 * h : 64 * h + 64, qo : qo + 512],
                            start=True,
                            stop=True,
                        )
                    at = attnp.tile([128, 1024], bf16)
                    nc.scalar.activation(
                        at[:, off:1024], sp[:, off:1024], AF.Exp, scale=SCALE
                    )
                    if j >= 0:
                        # causal mask reduces to a [128,128] triangle on the
                        # q-subrange [off, off+128) of each head
                        for h in range(2):
                            c = 512 * h + off
                            nc.vector.tensor_mul(
                                at[:, c : c + 128], at[:, c : c + 128], tri_sb[:]
                            )
                    for h in range(2):
                        nc.tensor.matmul(
                            zp[h][:, off:512],
                            v_sb[2 * hp + h][:, kt, 0:65],
                            at[:, 512 * h + off : 512 * h + 512],
                            start=(kt == 0),
                            stop=(kt == nkt - 1),
                        )
                    if kt == 1 or kt == 5:
                        pump_norm()
                    elif kt % 2 == 1 and not last:
                        pump_op()
                # evacuate z + sums row to SBUF so PSUM banks free up; a
                # small DMA gathers the two sums rows onto partitions {0,1}
                zsU = [
                    small.tile([65, 512], bf16, tag=f"zsU{h}", name=f"zsU{h}")
                    for h in range(2)
                ]
                rsin = small.tile([2, 512], bf16, tag="rsin")
                rs2 = small.tile([2, 512], bf16, tag="rs2")
                for h in range(2):
                    nc.vector.tensor_copy(zsU[h][:], zp[h][:])
                    # gather the sums rows via the idle GpSimd (SWDGE) queue
                    # so they never wait behind sync-queue transposes; the
                    # last unit uses the (by then idle) low-latency HWDGE
                    eng = nc.sync if last else nc.gpsimd
                    eng.dma_start(rsin[h : h + 1, :], zsU[h][64:65, :])
                st = (qb, hp, zsU, rsin, rs2)
                normq.append(lambda st=st: norm_stage_a(st))

                def _stage_b(st=st, qb=qb, hp=hp):
                    norm_stage_b(st)
                    normb_done.add((qb, hp))

                normq.append(_stage_b)
                if hp == 1 and qb < NQB - 1:
                    for qx in range(4):
                        opq.append(
                            ((qb, 1), lambda qtg=4 * qb + qx: emit_outproj_qt(qtg))
                        )


            # ---- master schedule: QKV blocks interleaved with attention
            # units as dependencies allow, so the PE-dense QKV stream and
            # the ACT-dense softmax stream overlap.
            emit_qkv(0, 0, half_major=True)
            emit_qkv(0, 1)
            emit_unit(0, 0)
            emit_qkv(1, 0)
            emit_unit(0, 1)
            emit_unit(1, 0)
            emit_qkv(1, 1)
            emit_unit(1, 1)
            emit_unit(2, 0)
            emit_unit(2, 1)
            emit_unit(3, 0)
            emit_unit(3, 1, last=True)

            # drain the tail: last norms, then the last query block's
            # output projections with evac split across ACT and DVE
            while opq:
                opq.popleft()[1]()
            while normq:
                normq.popleft()()
            for qx in range(4):
                emit_outproj_qt(12 + qx, tail=True)

    return nc


@functools.lru_cache(maxsize=1)
def _get_nc():
    _install_waitfix()
    return _build_nc()


def _to_bf16(a):
    return np.ascontiguousarray(np.asarray(a, dtype=np.float32)).astype(
        ml_dtypes.bfloat16
    )


def _prepare_in_maps(
    normalized_resid_pre, W_Q, W_K, W_V, W_O, b_Q, b_K, b_V, b_O
):
    x = np.asarray(normalized_resid_pre, dtype=np.float32)
    W_Q = np.asarray(W_Q, dtype=np.float32)
    W_K = np.asarray(W_K, dtype=np.float32)
    W_V = np.asarray(W_V, dtype=np.float32)
    W_O = np.asarray(W_O, dtype=np.float32)
    b_Q = np.asarray(b_Q, dtype=np.float32)
    b_K = np.asarray(b_K, dtype=np.float32)
    b_V = np.asarray(b_V, dtype=np.float32)
    b_O = np.asarray(b_O, dtype=np.float32)

    xT_b = [_to_bf16(x[b].T) for b in range(B)]

    kk = np.arange(128)[:, None]
    qq = np.arange(128)[None, :]
    tri_np = (kk <= qq).astype(np.float32).astype(ml_dtypes.bfloat16)

    ones_np = np.zeros((2, 128), np.float32)
    ones_np[0, :64] = 1.0
    ones_np[1, 64:] = 1.0
    ones_np = ones_np.astype(ml_dtypes.bfloat16)

    in_maps = []
    for c in range(NCORES):
        b = c // 4
        h0 = 4 * (c % 4)
        cols = []
        bcols = []
        for hp in range(NHP):
            ha, hb = h0 + 2 * hp, h0 + 2 * hp + 1
            cols += [W_Q[ha], W_Q[hb], W_K[ha], W_K[hb], W_V[ha], W_V[hb]]
            bcols += [
                np.concatenate([b_Q[ha], b_Q[hb]]),
                np.concatenate([b_K[ha], b_K[hb]]),
                np.concatenate([b_V[ha], b_V[hb]]),
            ]
        wqkv_c = np.concatenate(cols, axis=1)  # [1024, 768]
        # pre-swizzle to the SBUF-linear layout [p, hp, g, a, 128] so the
        # weight loads are contiguous (V slice first for fast startup)
        wqkv_c = (
            wqkv_c.reshape(NDC, 128, NHP, 3, 128)
            .transpose(1, 2, 3, 0, 4)
            .reshape(128, NHP * NDC * 384)
        )
        bqkv_c = np.stack(bcols, axis=1).astype(np.float32)  # [128, 6]
        wo_c = np.concatenate(
            [
                np.concatenate([W_O[h0], W_O[h0 + 1]], axis=0),
                np.concatenate([W_O[h0 + 2], W_O[h0 + 3]], axis=0),
            ],
            axis=1,
        )  # [128, 2048]
        in_maps.append(
            {
                "xT": xT_b[b],
                "wqkv": _to_bf16(wqkv_c),
                "bqkv": np.ascontiguousarray(bqkv_c),
                "wo": _to_bf16(wo_c),
                "tri": tri_np,
                "ones1": ones_np,
            }
        )
    return in_maps, b_O


def _gather(res, b_O):
    out = np.zeros((B, SC, D), np.float32)
    for c, r in enumerate(res.results):
        out[c // 4] += r["outp"].astype(np.float32)
    out += b_O[None, None, :]
    return out


def kernel(
    normalized_resid_pre, W_Q, W_K, W_V, W_O, b_Q, b_K, b_V, b_O, **_unused
):
    in_maps, b_O = _prepare_in_maps(
        normalized_resid_pre, W_Q, W_K, W_V, W_O, b_Q, b_K, b_V, b_O
    )
    nc = _get_nc()
    res = run_bass_kernel_spmd(nc, in_maps, core_ids=list(range(NCORES)))
    return _gather(res, b_O)


def _try_install_profhook():
    """Register the axon NTFF profile hook (the container's antenv stub
    lacks axon_hooks); harmless no-op if anything is missing."""
    try:
        import sys
        import types

        if "antenv.axon_hooks" not in sys.modules:
            mod = types.ModuleType("antenv.axon_hooks")
            hook = [None]
            mod.set_axon_ntff_profile_hook = lambda h: hook.__setitem__(0, h)
            mod.get_axon_ntff_profile_hook = lambda: hook[0]
            sys.modules["antenv.axon_hooks"] = mod
            import antenv

            antenv.axon_hooks = mod
            from trn_agent_boot.trn_boot import _ntff_profile_via_ctypes

            mod.set_axon_ntff_profile_hook(
                _ntff_profile_via_ctypes("/opt/axon/libaxon_pjrt.so")
            )
            import concourse.bass_utils as bu

            bu.upload_artifacts = lambda tmpdir: f"file://{tmpdir}"
    except Exception:
        pass


def kernel_profiled(**inputs):
    """Like kernel() but with NTFF tracing; returns (out, BassKernelResults)."""
    _try_install_profhook()
    inputs = {k: v for k, v in inputs.items()}
    in_maps, b_O = _prepare_in_maps(
        inputs["normalized_resid_pre"],
        inputs["W_Q"],
        inputs["W_K"],
        inputs["W_V"],
        inputs["W_O"],
        inputs["b_Q"],
        inputs["b_K"],
        inputs["b_V"],
        inputs["b_O"],
    )
    nc = _get_nc()
    res = run_bass_kernel_spmd(
        nc, in_maps, core_ids=list(range(NCORES)), trace=True
    )
    return _gather(res, b_O), res


if __name__ == "__main__":
    rng = np.random.default_rng(0)
    inputs = {
        "normalized_resid_pre": rng.standard_normal((B, S, D)).astype(np.float32),
        "W_Q": (rng.standard_normal((NHEAD, D, HDIM)) * 0.02).astype(np.float32),
        "W_K": (rng.standard_normal((NHEAD, D, HDIM)) * 0.02).astype(np.float32),
        "W_V": (rng.standard_normal((NHEAD, D, HDIM)) * 0.02).astype(np.float32),
        "W_O": (rng.standard_normal((NHEAD, HDIM, D)) * 0.02).astype(np.float32),
        "b_Q": np.zeros((NHEAD, HDIM), np.float32),
        "b_K": np.zeros((NHEAD, HDIM), np.float32),
        "b_V": np.zeros((NHEAD, HDIM), np.float32),
        "b_O": np.zeros((D,), np.float32),
    }
    out = kernel(**inputs)
    print("out", out.shape, out.dtype, float(np.abs(out).max()))
